# revision 19
# baseline (speedup 1.0000x reference)
"""Trainium2 Bass kernel: CrossAttnBlock (16x4096x512 query, 16x77x768 cond).

Sharding: pure data-parallel over batch -- 2 batches per core on 8 cores,
no collectives.  Host-side work is layout-only (transposes / slicing / bf16
cast).

On-device dataflow per core (activations kept transposed: feature dim on
SBUF partitions, tokens on the free dim; all matmul inputs bf16, PSUM f32):
    qT = wqT-stationary matmuls over xT chunks  [512f x 512t] per chunk
    kT = wkT-stationary matmuls over condT      [512f x 154s]
    v  = condT-stationary matmuls               [77s x 512d] -> v_aug [77, 8*65]
    scoresT pair p = kT_p row-halves vs qT_p row-halves -> one [77, 1024]
        2-bank PSUM tile; the two matmuls are emitted back-to-back so the
        PE runs them concurrently in the two 64-row groups (~339ns/pair).
    e_p = exp(scoresT_p / 8)   one ACT op per pair [77, 1024]
    avT_h = v_aug_h @ e slice  [65, 512t] (row 64 = softmax denominator)
    denominator rows DMA-gathered into [8, 512]; DVE cast + approx-recip;
    DRAM bounce + 16 partition-split stride-0 replicate DMAs materialize
    bc [128, 4*CHUNK] (both parities); norm rows 0:64 multiplied on GPSIMD,
    rows 64:128 on DVE.
    yT = woT-stationary matmuls over norm column-slices; evac + DMA out

PSUM budget (8 banks): scores 2 tiles x 2 banks, Q/O-proj shared pool
2 banks, AV 2 banks.  Emission is software-pipelined so each engine's
in-order stream has its cross-engine dependencies already satisfied:
    iter i:  scores+exp(i) | Qproj(i+1) | dma(i+2) | AV+norm(i-1) | Oproj(i-2)
with scores pairs and Q f-tiles interleaved at pair granularity, and AV
head-pairs interleaved with O-proj f-tiles.  Initial weight/x DMAs are
split 4-ways per tile across rings to cut the startup ramp.
"""

import os
import numpy as np

MODEL_DIM = 512
COND_DIM = 768
HEAD_DIM = 64
N_HEADS = 8
B = 16
T = 4096
LK = 77
N_CORES = 8
NB = B // N_CORES          # batches per core
CHUNK = 512                # tokens per chunk
NCHUNK = T // CHUNK
NCTOT = NB * NCHUNK        # total chunks per core
KD = MODEL_DIM // 128      # 4 partition tiles of model dim
CDT = COND_DIM // 128      # 6 partition tiles of cond dim
NP = N_HEADS // 2          # head pairs
SCALE = HEAD_DIM ** -0.5

_PROG = None               # cached compiled Bass program
LAST_RESULTS = None        # BassKernelResults of last run (for profiling)


def _build_program():
    import concourse.bass as bass  # noqa: F401
    import concourse.tile as tile
    from concourse import bacc, mybir
    from contextlib import ExitStack

    f32 = mybir.dt.float32
    bf16 = mybir.dt.bfloat16
    Exp = mybir.ActivationFunctionType.Exp

    nc = bacc.Bacc(
        "TRN2", target_bir_lowering=False, debug=False, num_devices=N_CORES
    )

    xt = nc.dram_tensor("xt", [NB, MODEL_DIM, T], bf16, kind="ExternalInput").ap()
    condt = nc.dram_tensor(
        "condt", [COND_DIM, NB * LK], bf16, kind="ExternalInput"
    ).ap()
    wqt = nc.dram_tensor("wqt", [MODEL_DIM, MODEL_DIM], bf16, kind="ExternalInput").ap()
    wkt = nc.dram_tensor("wkt", [COND_DIM, MODEL_DIM], bf16, kind="ExternalInput").ap()
    wvt = nc.dram_tensor("wvt", [COND_DIM, MODEL_DIM], bf16, kind="ExternalInput").ap()
    wot = nc.dram_tensor("wot", [MODEL_DIM, MODEL_DIM], bf16, kind="ExternalInput").ap()
    yt = nc.dram_tensor("yt", [NB, MODEL_DIM, T], bf16, kind="ExternalOutput").ap()

    with tile.TileContext(nc) as tc, ExitStack() as ctx:
        wp = ctx.enter_context(tc.tile_pool(name="wp", bufs=1))
        bp = ctx.enter_context(tc.tile_pool(name="bp", bufs=1))   # per-batch stuff
        xp = ctx.enter_context(tc.tile_pool(name="xp", bufs=3))   # x chunks
        qp = ctx.enter_context(tc.tile_pool(name="qp", bufs=3))   # qT chunks
        epool = ctx.enter_context(tc.tile_pool(name="epool", bufs=2))
        avp = ctx.enter_context(tc.tile_pool(name="avp", bufs=2))  # evac'd attnV
        rp = ctx.enter_context(tc.tile_pool(name="rp", bufs=2))    # denom rows
        rfp = ctx.enter_context(tc.tile_pool(name="rfp", bufs=2))  # 1/denom f32
        rbp = ctx.enter_context(tc.tile_pool(name="rbp", bufs=2))  # 1/denom bf16
        bcp = ctx.enter_context(tc.tile_pool(name="bcp", bufs=2))  # broadcasts
        drp = ctx.enter_context(tc.tile_pool(name="drp", bufs=2, space="DRAM"))
        npool = ctx.enter_context(tc.tile_pool(name="npool", bufs=3))
        yp = ctx.enter_context(tc.tile_pool(name="yp", bufs=2))
        psc = ctx.enter_context(tc.tile_pool(name="psc", bufs=2, space="PSUM"))
        pqo = ctx.enter_context(tc.tile_pool(name="pqo", bufs=2, space="PSUM"))
        pav = ctx.enter_context(tc.tile_pool(name="pav", bufs=2, space="PSUM"))

        # ---- weight / cond loads (split across DMA rings) ----
        def load_rows(pool, dram_ap, n_tiles, free, tagbase, split=1, eng=None):
            eng = eng or nc.sync
            tiles = []
            for k in range(n_tiles):
                t_ = pool.tile([128, free], bf16, tag=f"{tagbase}{k}",
                               name=f"{tagbase}{k}")
                step = 128 // split
                for s in range(split):
                    eng.dma_start(
                        out=t_[s * step:(s + 1) * step, :],
                        in_=dram_ap[k * 128 + s * step:k * 128 + (s + 1) * step, :],
                    )
                tiles.append(t_)
            return tiles

        # ---- pipeline state ----
        xt_sb = {}    # g -> list of 4 x tiles
        q_sb = {}     # g -> list of 4 qT tiles
        e_sb = {}     # g -> list of 4 exp pair tiles [77, 1024]
        av_sb = {}    # g -> [avsb0, avsb1] [65, 2048]
        norm_sb = {}  # g -> normalized avT tile [128, 4*CHUNK]

        def dma_load(g, split=1, eng=None):
            eng = eng or nc.sync
            b, t0 = g // NCHUNK, (g % NCHUNK) * CHUNK
            tiles = []
            for k in range(KD):
                xk = xp.tile([128, CHUNK], bf16, tag=f"xt{k}", name=f"x{g}_{k}")
                step = 128 // split
                for s in range(split):
                    eng.dma_start(
                        out=xk[s * step:(s + 1) * step, :],
                        in_=xt[b, k * 128 + s * step:k * 128 + (s + 1) * step,
                               t0:t0 + CHUNK],
                    )
                tiles.append(xk)
            xt_sb[g] = tiles

        def qproj_f(g, f):
            psq = pqo.tile([128, CHUNK], f32, tag="qo", name=f"psq{g}_{f}")
            for k in range(KD):
                nc.tensor.matmul(
                    psq,
                    lhsT=wq_sb[k][:, f * 128:(f + 1) * 128],
                    rhs=xt_sb[g][k],
                    start=(k == 0),
                    stop=(k == KD - 1),
                )
            qf = qp.tile([128, CHUNK], bf16, tag=f"q{f}", name=f"q{g}_{f}")
            if f == 0:
                nc.scalar.copy(qf, psq)
            else:
                nc.vector.tensor_copy(qf, psq)
            q_sb.setdefault(g, [None] * KD)[f] = qf
            if f == KD - 1:
                del xt_sb[g]

        def score_pair(g, p):
            """One [77, 1024] 2-bank PSUM tile; two row-half matmuls emitted
            back-to-back (PE runs them concurrently), then one exp."""
            b = g // NCHUNK
            pss = psc.tile([LK, 2 * CHUNK], f32, tag="s", name=f"pss{g}_{p}")
            for half in range(2):
                lo, hi = 64 * half, 64 * (half + 1)
                nc.tensor.matmul(
                    pss[:, half * CHUNK:(half + 1) * CHUNK],
                    lhsT=kt_sb[p][lo:hi, b * LK:(b + 1) * LK],
                    rhs=q_sb[g][p][lo:hi, :],
                    start=True,
                    stop=True,
                )
            e = epool.tile([LK, 2 * CHUNK], bf16, tag=f"e{p}", name=f"e{g}_{p}")
            nc.scalar.activation(e, pss, Exp, scale=SCALE)
            e_sb.setdefault(g, [None] * NP)[p] = e
            if p == NP - 1:
                del q_sb[g]

        def av_head(g, h):
            b = g // NCHUNK
            pavt = pav.tile([HEAD_DIM + 1, CHUNK], f32, tag="av",
                            name=f"pav{g}_{h}")
            e = e_sb[g][h // 2][:, (h % 2) * CHUNK:(h % 2 + 1) * CHUNK]
            nc.tensor.matmul(
                pavt,
                lhsT=v_aug[b][:, h * 65:(h + 1) * 65],
                rhs=e,
                start=True,
                stop=True,
            )
            return pavt

        def av_evac(g, h, pavt):
            avsb = av_sb[g][h % 2]
            dst = avsb[:, (h // 2) * CHUNK:(h // 2 + 1) * CHUNK]
            if h % 2 == 0:
                nc.scalar.copy(dst, pavt)
            else:
                nc.vector.tensor_copy(dst, pavt)

        def norm_chain(g):
            """denominator gather -> reciprocal -> DRAM bounce -> partition-
            split replicate DMAs -> two multiplies (gpsimd + vector)."""
            avsb = av_sb[g]
            df = rp.tile([N_HEADS, CHUNK], f32, tag="df", name=f"df{g}")
            for i in range(2):
                nc.gpsimd.dma_start(out=df[4 * i:4 * i + 4, :],
                                    in_=avsb[i][HEAD_DIM:HEAD_DIM + 1, :])
            rf = rfp.tile([N_HEADS, CHUNK], f32, tag="rf", name=f"rf{g}")
            nc.vector.reciprocal_approx_fast(out=rf, in_=df)
            scr = drp.tile([N_HEADS, CHUNK], bf16, tag="scr", name=f"scr{g}")
            nc.gpsimd.dma_start(out=scr, in_=rf)
            bcs = []
            for i in range(2):
                bc = bcp.tile([HEAD_DIM, KD * CHUNK], bf16, tag=f"bc{i}",
                              name=f"bc{g}_{i}")
                for p2 in range(2):
                    row = scr[0:1, :] if i == 0 else scr[4:5, :]
                    rep = bass.AP(row.tensor, row.offset,
                                  [[0, 32], [1, KD * CHUNK]])
                    nc.sync.dma_start(out=bc[32 * p2:32 * (p2 + 1), :], in_=rep)
                bcs.append(bc)
            norm = npool.tile([128, KD * CHUNK], bf16, tag="n", name=f"n{g}")
            nc.vector.tensor_mul(
                norm[0:HEAD_DIM, :], avsb[0][0:HEAD_DIM, :], bcs[0]
            )
            nc.vector.tensor_mul(
                norm[HEAD_DIM:128, :], avsb[1][0:HEAD_DIM, :], bcs[1]
            )
            del av_sb[g]
            del e_sb[g]
            norm_sb[g] = norm

        def oproj_f(g, f):
            psy = pqo.tile([128, CHUNK], f32, tag="qo", name=f"psy{g}_{f}")
            for j in range(KD):
                nc.tensor.matmul(
                    psy,
                    lhsT=wo_sb[j][:, f * 128:(f + 1) * 128],
                    rhs=norm_sb[g][:, j * CHUNK:(j + 1) * CHUNK],
                    start=(j == 0),
                    stop=(j == KD - 1),
                )
            return psy

        def oproj_store(g, f, psy):
            b, t0 = g // NCHUNK, (g % NCHUNK) * CHUNK
            ysb = yp.tile([128, CHUNK], bf16, tag=f"y{f}", name=f"y{g}_{f}")
            if f % 2 == 0:
                nc.scalar.copy(ysb, psy)
            else:
                nc.vector.tensor_copy(ysb, psy)
            nc.sync.dma_start(
                out=yt[b, f * 128:(f + 1) * 128, t0:t0 + CHUNK], in_=ysb
            )
            if f == KD - 1:
                del norm_sb[g]

        # ================= prologue =================
        # wq on the scalar HWDGE queue, x0 on sync -- issue in parallel.
        wq_sb = load_rows(wp, wqt, KD, MODEL_DIM, "wq", split=2, eng=nc.scalar)
        dma_load(0, split=2, eng=nc.sync)
        dma_load(1, split=1, eng=nc.sync)
        for f in range(KD):
            qproj_f(0, f)

        sp_setup = tc.alloc_tile_pool(name="sp_setup", bufs=1)
        wk_sb = load_rows(sp_setup, wkt, CDT, MODEL_DIM, "wk", eng=nc.scalar)
        cond_sb = []
        for k in range(CDT):
            t_ = sp_setup.tile([128, NB * LK], bf16, tag=f"cond{k}",
                               name=f"cond{k}")
            nc.sync.dma_start(out=t_, in_=condt[k * 128:(k + 1) * 128, :])
            cond_sb.append(t_)
        wv_sb = load_rows(sp_setup, wvt, CDT, MODEL_DIM, "wv", eng=nc.scalar)

        dma_load(2, split=1, eng=nc.sync)
        for f in range(KD):
            qproj_f(1, f)

        # ---- K projection (both batches at once): kT [512, NB*77] ----
        kt_sb = []
        for f in range(KD):
            psk = pqo.tile([128, NB * LK], f32, tag="qo", name=f"psk{f}")
            for c in range(CDT):
                nc.tensor.matmul(
                    psk,
                    lhsT=wk_sb[c][:, f * 128:(f + 1) * 128],
                    rhs=cond_sb[c],
                    start=(c == 0),
                    stop=(c == CDT - 1),
                )
            ktf = bp.tile([128, NB * LK], bf16, tag=f"kt{f}", name=f"kt{f}")
            nc.scalar.copy(ktf, psk)
            kt_sb.append(ktf)

        # ---- V projection per batch -> v_aug [77, 8*65] (65th col = ones) ----
        v_aug = []
        for b in range(NB):
            psv = pav.tile([LK, MODEL_DIM], f32, tag="av", name=f"psv{b}")
            for c in range(CDT):
                nc.tensor.matmul(
                    psv,
                    lhsT=cond_sb[c][:, b * LK:(b + 1) * LK],
                    rhs=wv_sb[c],
                    start=(c == 0),
                    stop=(c == CDT - 1),
                )
            va = bp.tile([LK, N_HEADS * (HEAD_DIM + 1)], bf16, tag=f"va{b}",
                         name=f"va{b}")
            for h in range(N_HEADS):
                nc.scalar.copy(
                    va[:, h * 65:h * 65 + 64], psv[:, h * 64:(h + 1) * 64]
                )
            ones_view = va.rearrange("p (h c) -> p h c", c=65)[:, :, 64]
            nc.vector.memset(ones_view, 1.0)
            v_aug.append(va)

        wo_sb = load_rows(wp, wot, KD, MODEL_DIM, "wo", eng=nc.scalar)
        sp_setup.release()

        # ================= steady-state loop =================
        # iteration i: scores+exp(i) | qproj(i+2) | dma(i+3) | AV+norm(i-1)
        #              | oproj+store(i-3)
        for i in range(NCTOT + 3):
            sc, qpi, avx, op, dm = i, i + 2, i - 1, i - 3, i + 3

            def alloc_av(g):
                if g not in av_sb:
                    av_sb[g] = [
                        avp.tile([HEAD_DIM + 1, KD * CHUNK], bf16, tag=f"av{j}",
                                 name=f"avsb{g}_{j}")
                        for j in range(2)
                    ]

            # Fully staggered schedule: the 4 scores pairs are spread
            # through the iteration (pair p ~2.3us apart) so the scalar
            # engine's exps never gate the PE, and every PSUM-slot WAR has
            # >0.3us of slack.  PE order:
            #   pairA Qf0 pairB Qf1 | Of0 AV01 pairC Qf2 | Of1 AV23 pairD
            #   Qf3 | Of2 AV45 Of3 AV67
            if 0 <= avx < NCTOT:
                alloc_av(avx)

            def av_pair(f):
                h0, h1 = 2 * f, 2 * f + 1
                pava = av_head(avx, h0)
                pavb = av_head(avx, h1)
                av_evac(avx, h0, pava)
                av_evac(avx, h1, pavb)

            if sc < NCTOT:
                score_pair(sc, 0)
            if qpi < NCTOT:
                qproj_f(qpi, 0)
            if sc < NCTOT:
                score_pair(sc, 1)
            if qpi < NCTOT:
                qproj_f(qpi, 1)
            if 0 <= op < NCTOT:
                psy = oproj_f(op, 0)
            if 0 <= avx < NCTOT:
                av_pair(0)
            if 0 <= op < NCTOT:
                oproj_store(op, 0, psy)
            if sc < NCTOT:
                score_pair(sc, 2)
            if qpi < NCTOT:
                qproj_f(qpi, 2)
            if 0 <= op < NCTOT:
                psy = oproj_f(op, 1)
            if 0 <= avx < NCTOT:
                av_pair(1)
            if 0 <= op < NCTOT:
                oproj_store(op, 1, psy)
            if sc < NCTOT:
                score_pair(sc, 3)
            if qpi < NCTOT:
                qproj_f(qpi, 3)
            if 0 <= op < NCTOT:
                psy = oproj_f(op, 2)
            if 0 <= avx < NCTOT:
                av_pair(2)
            if 0 <= op < NCTOT:
                oproj_store(op, 2, psy)
            if 0 <= op < NCTOT:
                psy = oproj_f(op, 3)
            if 0 <= avx < NCTOT:
                av_pair(3)
            if 0 <= op < NCTOT:
                oproj_store(op, 3, psy)
            if dm < NCTOT:
                dma_load(dm)
            if 0 <= avx < NCTOT:
                norm_chain(avx)

    nc.compile()
    return nc


def _get_program():
    global _PROG
    if _PROG is None:
        _PROG = _build_program()
    return _PROG


def _shard_inputs(x, cond, w_q, w_k, w_v, w_o):
    """Host-side layout: transpose + shard + bf16 cast. Returns per-core in_maps."""
    import ml_dtypes

    bf = ml_dtypes.bfloat16
    x = np.ascontiguousarray(x, dtype=bf)
    cond = np.ascontiguousarray(cond, dtype=bf)
    wqt = np.ascontiguousarray(w_q.T.astype(bf))
    wkt = np.ascontiguousarray(w_k.T.astype(bf))
    wvt = np.ascontiguousarray(w_v.T.astype(bf))
    wot = np.ascontiguousarray(w_o.T.astype(bf))

    xT = np.ascontiguousarray(x.transpose(0, 2, 1))          # [B, D, T]
    condT = np.ascontiguousarray(cond.transpose(0, 2, 1))    # [B, CD, LK]

    in_maps = []
    for c in range(N_CORES):
        b0 = c * NB
        ct = np.ascontiguousarray(
            condT[b0:b0 + NB].transpose(1, 0, 2).reshape(COND_DIM, NB * LK)
        )
        in_maps.append(
            {
                "xt": np.ascontiguousarray(xT[b0:b0 + NB]),
                "condt": ct,
                "wqt": wqt,
                "wkt": wkt,
                "wvt": wvt,
                "wot": wot,
            }
        )
    return in_maps


def kernel(x, cond, w_q, w_k, w_v, w_o):
    global LAST_RESULTS
    from concourse.bass_utils import run_bass_kernel_spmd

    nc = _get_program()
    in_maps = _shard_inputs(x, cond, w_q, w_k, w_v, w_o)
    trace = bool(os.environ.get("BASS_TRACE"))
    res = run_bass_kernel_spmd(
        nc, in_maps, list(range(N_CORES)), trace=trace
    )
    LAST_RESULTS = res

    out = np.empty((B, T, MODEL_DIM), dtype=np.float32)
    for c in range(N_CORES):
        ytc = np.asarray(res.results[c]["yt"], dtype=np.float32)   # [NB, D, T]
        out[c * NB:(c + 1) * NB] = ytc.transpose(0, 2, 1)
    return out
